# revision 7
# baseline (speedup 1.0000x reference)
"""Trainium2 Bass kernel for AttentionTopK (B=128, N=512, D=256, K=8).

Math (reference, with mask == all-ones which is the only supported case):
    xs    = x / sqrt(D)
    sims  = xs @ xs.T per batch          [N, N], diag excluded
    idx   = top-8 neighbours per row
    attn  = sum of the 8 neighbour rows of xs, / 8
    out   = attn @ W.T + b

Device formulation (per batch element):
    S     = x @ x.T                      (symmetric; top-k is scale-invariant)
    S    += -1e30 on the diagonal
    t[n]  = 8th largest of row n         (one Max8 pass per 128-row tile)
    Sel[n, m] = S[n, m] >= t[n]          (tensor_scalar, per-partition threshold)
    SelT  = Sel.T                        (PE pass-through transposes; 0/1 exact)
    y     = x @ W.T
    out   = (SelT.T @ y) / (16 * 8) + b  (16 = sqrt(D), 8 = denom)

Sharding: batch dim 128 -> 16 per core across 8 cores.
"""

import os

import numpy as np

B, N, D = 128, 512, 256
NCORES = 8
BPC = B // NCORES  # batches per core
NT = N // 128      # n tiles of 128 rows
DC = D // 128      # d chunks of 128

# matmul input dtype knobs: "f32r" (full-rate, reduced precision) or "f32"
# (4x slower). SIMS_DT covers x/W operands (sims + y matmuls); OUT_DT covers
# the selection matmul operands (Sel is exact 0/1 at any precision).
SIMS_DT = os.environ.get("K_SIMS_DT", "f32")
OUT_DT = os.environ.get("K_OUT_DT", "f32r")

_CACHE: dict = {}


def _mm_dt(name):
    import concourse.mybir as mybir

    return {"f32r": mybir.dt.float32r, "f32": mybir.dt.float32}[name]


def _build_program(include_bias: bool):
    import concourse.mybir as mybir
    import concourse.tile as tile
    from concourse import bacc

    f32 = mybir.dt.float32
    mm_s = _mm_dt(SIMS_DT)
    mm_o = _mm_dt(OUT_DT)

    nc = bacc.Bacc("TRN2", target_bir_lowering=False, debug=False)

    x_d = nc.dram_tensor("x", [BPC, N, D], f32, kind="ExternalInput").ap()
    wt_d = nc.dram_tensor("wt", [D, D], f32, kind="ExternalInput").ap()
    dneg_d = nc.dram_tensor("dneg", [128, 128], f32, kind="ExternalInput").ap()
    ident_d = nc.dram_tensor("ident", [128, 128], f32, kind="ExternalInput").ap()
    if include_bias:
        bb_d = nc.dram_tensor("bb", [128, D], f32, kind="ExternalInput").ap()
    out_d = nc.dram_tensor("out", [BPC, N, D], f32, kind="ExternalOutput").ap()

    with tile.TileContext(nc) as tc:
        with (
            tc.tile_pool(name="const", bufs=1) as cpool,
            tc.tile_pool(name="sb", bufs=2) as sb,
            tc.tile_pool(name="ps_xt", bufs=1, space="PSUM") as ps_xt,
            tc.tile_pool(name="ps_s", bufs=2, space="PSUM") as ps_s,
            tc.tile_pool(name="ps_sel", bufs=2, space="PSUM") as ps_sel,
            tc.tile_pool(name="ps_y", bufs=2, space="PSUM") as ps_y,
            tc.tile_pool(name="ps_o", bufs=1, space="PSUM") as ps_o,
        ):
            wt_raw = cpool.tile([128, DC, D], f32)
            for dc in range(DC):
                nc.sync.dma_start(out=wt_raw[:, dc, :], in_=wt_d[128 * dc : 128 * (dc + 1), :])
            wt_sb = cpool.tile([128, DC, D], mm_o)
            nc.scalar.copy(out=wt_sb, in_=wt_raw)
            dneg_sb = cpool.tile([128, 128], f32)
            nc.sync.dma_start(out=dneg_sb, in_=dneg_d)
            ident_sb = cpool.tile([128, 128], f32)
            nc.sync.dma_start(out=ident_sb, in_=ident_d)
            ident_o = cpool.tile([128, 128], mm_o)
            nc.scalar.copy(out=ident_o, in_=ident_sb)
            if include_bias:
                bb_sb = cpool.tile([128, D], f32)
                nc.sync.dma_start(out=bb_sb, in_=bb_d)

            for b in range(BPC):
                # ---- load x[b] as [128, NT, D] (row tile t on partition p = row 128t+p)
                xb = sb.tile([128, NT, D], f32, tag="xb")
                for t in range(NT):
                    nc.sync.dma_start(
                        out=xb[:, t, :], in_=x_d[b, 128 * t : 128 * (t + 1), :]
                    )

                # ---- transpose to xT [d, n]: xt_sb[p, dc, n] = x[n, 128*dc + p]
                xt_sb = sb.tile([128, DC, N], mm_s, tag="xt")
                if SIMS_DT == OUT_DT:
                    xt_o = xt_sb
                else:
                    xt_o = sb.tile([128, DC, N], mm_o, tag="xto")
                for dc in range(DC):
                    pxt = ps_xt.tile([128, N], f32, tag="pxt")
                    for t in range(NT):
                        nc.tensor.transpose(
                            out=pxt[:, 128 * t : 128 * (t + 1)],
                            in_=xb[:, t, 128 * dc : 128 * (dc + 1)],
                            identity=ident_sb,
                        )
                    nc.scalar.copy(out=xt_sb[:, dc, :], in_=pxt)
                    if xt_o is not xt_sb:
                        nc.scalar.copy(out=xt_o[:, dc, :], in_=pxt)

                # ---- S row tiles: matmul -> diag mask -> max8 -> select (all on PSUM)
                m8 = sb.tile([128, NT * 8], f32, tag="m8")
                sel_n = sb.tile([128, NT, N], mm_o, tag="sel_n")
                for i in range(NT):
                    ps = ps_s.tile([128, N], f32, tag="ps")
                    for dc in range(DC):
                        nc.tensor.matmul(
                            out=ps,
                            lhsT=xt_sb[:, dc, 128 * i : 128 * (i + 1)],
                            rhs=xt_sb[:, dc, :],
                            start=(dc == 0),
                            stop=(dc == DC - 1),
                        )
                    # exclude self: diagonal block gets -1e30 (in-place in PSUM)
                    nc.vector.tensor_add(
                        out=ps[:, 128 * i : 128 * (i + 1)],
                        in0=ps[:, 128 * i : 128 * (i + 1)],
                        in1=dneg_sb,
                    )
                    nc.vector.max(out=m8[:, 8 * i : 8 * (i + 1)], in_=ps)
                    # Sel[n, m] = S[n, m] >= (8th largest of row n)
                    nc.vector.tensor_scalar(
                        out=sel_n[:, i, :],
                        in0=ps,
                        scalar1=m8[:, 8 * i + 7 : 8 * i + 8],
                        scalar2=None,
                        op0=mybir.AluOpType.is_ge,
                    )

                # ---- SelT = Sel.T via 16 pass-through block transposes (0/1 exact)
                selT = sb.tile([128, NT, N], mm_o, tag="selT")
                for j in range(NT):
                    psl = ps_sel.tile([128, N], mm_o, tag="psl")
                    for i in range(NT):
                        nc.tensor.transpose(
                            out=psl[:, 128 * i : 128 * (i + 1)],
                            in_=sel_n[:, i, 128 * j : 128 * (j + 1)],
                            identity=ident_o,
                        )
                    nc.scalar.copy(out=selT[:, j, :], in_=psl)

                # ---- y = x @ W.T
                y_sb = sb.tile([128, NT, D], mm_o, tag="y")
                for i in range(NT):
                    py = ps_y.tile([128, D], f32, tag="py")
                    for dc in range(DC):
                        nc.tensor.matmul(
                            out=py,
                            lhsT=xt_o[:, dc, 128 * i : 128 * (i + 1)],
                            rhs=wt_sb[:, dc, :],
                            start=(dc == 0),
                            stop=(dc == DC - 1),
                        )
                    nc.scalar.copy(out=y_sb[:, i, :], in_=py)

                # ---- out = (SelT.T @ y) / 128 (+ b), store
                out_sb = sb.tile([128, NT, D], f32, tag="osb")
                for i in range(NT):
                    po = ps_o.tile([128, D], f32, tag="po")
                    for j in range(NT):
                        nc.tensor.matmul(
                            out=po,
                            lhsT=selT[:, j, 128 * i : 128 * (i + 1)],
                            rhs=y_sb[:, j, :],
                            start=(j == 0),
                            stop=(j == NT - 1),
                        )
                    nc.scalar.mul(out=out_sb[:, i, :], in_=po, mul=1.0 / 128.0)
                    if include_bias:
                        nc.vector.tensor_add(
                            out=out_sb[:, i, :], in0=out_sb[:, i, :], in1=bb_sb
                        )
                    nc.sync.dma_start(
                        out=out_d[b, 128 * i : 128 * (i + 1), :], in_=out_sb[:, i, :]
                    )

    nc.compile()
    return nc


def _get_program(include_bias: bool):
    key = (include_bias, SIMS_DT, OUT_DT)
    if key not in _CACHE:
        _CACHE[key] = _build_program(include_bias)
    return _CACHE[key]


def _consts():
    dneg = np.where(np.eye(128, dtype=bool), np.float32(-1e30), np.float32(0.0)).astype(
        np.float32
    )
    ident = np.eye(128, dtype=np.float32)
    return dneg, ident


def _in_maps(x, W, b, include_bias):
    dneg, ident = _consts()
    wt = np.ascontiguousarray(W.T.astype(np.float32))
    maps = []
    for c in range(NCORES):
        m = {
            "x": np.ascontiguousarray(x[c * BPC : (c + 1) * BPC]),
            "wt": wt,
            "dneg": dneg,
            "ident": ident,
        }
        if include_bias:
            m["bb"] = np.ascontiguousarray(
                np.broadcast_to(b.astype(np.float32), (128, D)).copy()
            )
        maps.append(m)
    return maps


def _run(x, mask, W, b, trace=False):
    from concourse.bass_utils import run_bass_kernel_spmd

    x = np.asarray(x, dtype=np.float32)
    mask = np.asarray(mask)
    W = np.asarray(W, dtype=np.float32)
    b = np.asarray(b, dtype=np.float32)
    assert x.shape == (B, N, D), x.shape
    assert bool(mask.all()), "kernel supports the all-ones mask only"

    include_bias = bool(np.any(b))
    nc = _get_program(include_bias)
    maps = _in_maps(x, W, b, include_bias)
    res = run_bass_kernel_spmd(nc, maps, core_ids=list(range(NCORES)), trace=trace)
    out = np.concatenate([r["out"] for r in res.results], axis=0)
    return out, res


def kernel(x, mask, W, b):
    out, _ = _run(x, mask, W, b, trace=False)
    return out
